# revision 2
# baseline (speedup 1.0000x reference)
"""DigitCapsuleLayer forward (2 routing iterations) on 8 Trainium2 cores — V2.1.

d-major restructure: m-rows ordered m = d*N + n so the routing logits
Delta[n,b] dedupe 8x for the sigmoid and the d-fold is cheap DVE adds.

Per core (32 batches):
  phase1:  S[b,je]  = sum_m x[m,b]*Wf[m,je]     (432 PE matmuls; Wf pre-scaled
                                                 by 0.5 host-side, out rescaled)
  squash:  v1 = squash(S'); vtil = [v1_0, -v1_1]; vtT [32je, 32b]
  g:       psg-tile[m128, b32] = WfT-tile^T @ vtT   (432 matmuls, 32-contract)
  T1:      tch_d = psg_d * x_d    (Pool direct / Act copy + DVE mul per d)
  T2:      Delta = sum_d tch_d    (DVE adds, tree, interleaved)
  sigma:   cbf = sigmoid(Delta)   (Act, [128,1728] once)
  T4:      ybf_d = cbf * x_d      (DVE/Pool, overwrites tch)
  s2:      A[b,je] = sum_m ybf[m,b]*Wf[m,je]    (432 PE matmuls)
  out:     s2_j0 = A_j0 ; s2_j1 = S_j1 - A_j1 ; (x2 rescale + squash host-side)
"""

import os
os.environ.setdefault("NEURON_RT_RESET_CORES", "1")

import numpy as np
import ml_dtypes

import concourse.bacc as bacc
import concourse.mybir as mybir
import concourse.tile as tile
from concourse.bass_utils import run_bass_kernel_spmd

B = 256
NCORES = 8
BC = B // NCORES          # 32 batch per core
N = 6912
D = 8
E = 16
J = 2
JE = J * E                # 32
NBT = N // 128            # 54 tiles per d-slice
NT = D * NBT              # 432 tiles total
M = NT * 128              # 55296
HT = 27                   # tiles per half-d-slice
HSL = HT * BC             # 864 free per half-d-slice
SLICE = NBT * BC          # 1728 free per d-slice
FREE = NT * BC            # 13824
EPS = 1e-9

BF16 = mybir.dt.bfloat16
F8 = mybir.dt.float8e4
F32 = mybir.dt.float32

POOL_DIRECT = ()             # (GPSIMD cannot access PSUM on HW)
POOL_T4 = (1, 6)             # d-slices whose T4 mul runs on GpSimd

_cached = None


def _build_program():
    nc = bacc.Bacc("TRN2", num_devices=NCORES)

    xt = nc.dram_tensor("xt", [128, FREE], BF16, kind="ExternalInput")
    wf = nc.dram_tensor("wf", [128, FREE], BF16, kind="ExternalInput")
    wft = nc.dram_tensor("wft", [32, M], F8, kind="ExternalInput")
    vout = nc.dram_tensor("vout", [BC, JE], F32, kind="ExternalOutput")

    with tile.TileContext(nc) as tc:
        with (
            tc.tile_pool(name="big", bufs=1) as big,
            tc.tile_pool(name="small", bufs=1) as small,
            tc.tile_pool(name="tchp", bufs=1) as tchp,
            tc.tile_pool(name="ps_acc", bufs=1, space="PSUM") as ps_acc,
            tc.tile_pool(name="ps_g", bufs=3, space="PSUM") as ps_g,
        ):
            XT = big.tile([128, FREE], BF16, tag="XT")
            WF = big.tile([128, FREE], BF16, tag="WF")
            WFT = big.tile([32, M], F8, tag="WFT")

            # ---- warm the Sqrt act table early ----
            warm = small.tile([BC, J], F32, tag="warm")
            nc.vector.memset(warm[:], 0.0)
            nc.scalar.activation(warm[:], warm[:],
                                 mybir.ActivationFunctionType.Sqrt)

            # ---- DMA: xt+wf interleaved by d-slice, then wft ----
            for d in range(D):
                lo, hi = d * SLICE, (d + 1) * SLICE
                nc.sync.dma_start(XT[:, lo:hi], xt[:, lo:hi])
                nc.sync.dma_start(WF[:, lo:hi], wf[:, lo:hi])
            for i in range(D):
                nc.sync.dma_start(WFT[:, i * N:(i + 1) * N],
                                  wft[:, i * N:(i + 1) * N])

            # ---- phase 1: S = x^T Wf', accumulate over all 432 tiles ----
            ps1 = ps_acc.tile([BC, JE], F32, tag="psacc")
            for t in range(NT):
                nc.tensor.matmul(
                    ps1[:],
                    lhsT=XT[:, t * BC:(t + 1) * BC],
                    rhs=WF[:, t * JE:(t + 1) * JE],
                    start=(t == 0),
                    stop=(t == NT - 1),
                )

            # ---- squash(S') -> v1 -> vtil -> vtT  (Wf carries the 0.5) ----
            S = small.tile([BC, JE], F32, tag="S")
            sq = small.tile([BC, JE], F32, tag="sq")
            n2 = small.tile([BC, J], F32, tag="n2")
            d1 = small.tile([BC, J], F32, tag="d1")
            q = small.tile([BC, J], F32, tag="q")
            p2 = small.tile([BC, J], F32, tag="p2")
            r = small.tile([BC, J], F32, tag="r")
            f = small.tile([BC, J], F32, tag="f")
            vt = small.tile([BC, JE], BF16, tag="vt")
            vtT = small.tile([BC, JE], BF16, tag="vtT")

            # critical chain kept short: everything reads ps1/PSUM directly;
            # the S copy (needed only for the final s2 assembly) is off-chain.
            nc.vector.tensor_copy(S[:], ps1[:])
            nc.vector.tensor_mul(sq[:], S[:], S[:])
            nc.vector.reduce_sum(
                n2[:], sq.rearrange("p (j e) -> p j e", e=E),
                axis=mybir.AxisListType.X,
            )
            # f = +-n2 / ((1+n2) * sqrt(n2+eps)); sign folded into nn
            nn = small.tile([BC, J], F32, tag="nn")
            nc.vector.tensor_scalar_mul(nn[:, 0:1], n2[:, 0:1], 1.0)
            nc.vector.tensor_scalar_mul(nn[:, 1:2], n2[:, 1:2], -1.0)
            nc.vector.tensor_scalar_add(d1[:], n2[:], 1.0)
            nc.vector.tensor_scalar_add(q[:], n2[:], EPS)
            nc.vector.tensor_mul(p2[:], d1[:], d1[:])
            nc.vector.tensor_mul(p2[:], p2[:], q[:])
            nc.vector.reciprocal(r[:], p2[:])
            nc.scalar.activation(r[:], r[:], mybir.ActivationFunctionType.Sqrt)
            nc.vector.tensor_mul(f[:], nn[:], r[:])
            nc.vector.tensor_scalar_mul(vt[:, 0:E], ps1[:, 0:E], f[:, 0:1])
            nc.vector.tensor_scalar_mul(vt[:, E:JE], ps1[:, E:JE], f[:, 1:2])
            nc.vector.transpose(vtT[:], vt[:])
            # pre-load the Sigmoid act table now (depends on vtT so the
            # scheduler cannot float it ahead of the squash Sqrt)
            warm2 = small.tile([BC, JE], F32, tag="warm2")
            nc.scalar.activation(warm2[:], vtT[:],
                                 mybir.ActivationFunctionType.Sigmoid)

            # ---- routing: per half-d-slice g -> T1; T2 interleaved ----
            tch_l = {}

            def g_half(d, h):
                """g matmuls for half-slice (d,h) -> psum tile [128, 864]."""
                psg = ps_g.tile([128, HSL], F32, tag="psg")
                for i in range(HT):
                    t = d * NBT + h * HT + i
                    nc.tensor.matmul(
                        psg[:, i * BC:(i + 1) * BC],
                        lhsT=WFT[:, t * 128:(t + 1) * 128],
                        rhs=vtT[:],
                        start=True, stop=True,
                    )
                return psg

            def t1(d):
                lo = d * SLICE
                tch = tchp.tile([128, SLICE], BF16, tag=f"tch{d}")
                if d in POOL_DIRECT:
                    for h in range(2):
                        psg = g_half(d, h)
                        nc.gpsimd.tensor_mul(
                            tch[:, h * HSL:(h + 1) * HSL], psg[:],
                            XT[:, lo + h * HSL:lo + (h + 1) * HSL])
                else:
                    gbf = tchp.tile([128, SLICE], BF16, tag="gbf", bufs=2)
                    for h in range(2):
                        psg = g_half(d, h)
                        nc.scalar.copy(gbf[:, h * HSL:(h + 1) * HSL], psg[:])
                        nc.vector.tensor_mul(
                            tch[:, h * HSL:(h + 1) * HSL],
                            gbf[:, h * HSL:(h + 1) * HSL],
                            XT[:, lo + h * HSL:lo + (h + 1) * HSL])
                tch_l[d] = tch

            def t2_add(da, db, eng=None):
                (eng or nc.vector).tensor_add(tch_l[da][:], tch_l[da][:],
                                              tch_l[db][:])

            # interleave T1 and the T2 tree so adds run as pairs finish
            t1(0); t1(1)
            t2_add(0, 1)
            t1(2); t1(3)
            t2_add(2, 3)
            t1(4); t1(5)
            t2_add(4, 5); t2_add(0, 2)
            t1(6); t1(7)
            t2_add(6, 7); t2_add(4, 6); t2_add(0, 4)
            Delta = tch_l[0]

            # ---- sigmoid on distinct n only ----
            cbf = tchp.tile([128, SLICE], BF16, tag="cbf")
            nc.scalar.activation(cbf[:, 0:HSL], Delta[:, 0:HSL],
                                 mybir.ActivationFunctionType.Sigmoid)
            nc.scalar.activation(cbf[:, HSL:SLICE], Delta[:, HSL:SLICE],
                                 mybir.ActivationFunctionType.Sigmoid)

            # ---- T4 + s2: ybf_d = cbf * x_d ; A += ybf_d^T Wf_d ----
            ps2 = ps_acc.tile([BC, JE], F32, tag="psacc")

            for d in range(D):
                lo = d * SLICE
                ybf = tch_l[d]  # reuse; Delta (d=0) is dead after sigmoid
                eng = nc.gpsimd if d in POOL_T4 else nc.vector
                for h in range(2):
                    eng.tensor_mul(ybf[:, h * HSL:(h + 1) * HSL],
                                   cbf[:, h * HSL:(h + 1) * HSL],
                                   XT[:, lo + h * HSL:lo + (h + 1) * HSL])
                for nb in range(NBT):
                    t = d * NBT + nb
                    nc.tensor.matmul(
                        ps2[:],
                        lhsT=ybf[:, nb * BC:(nb + 1) * BC],
                        rhs=WF[:, t * JE:(t + 1) * JE],
                        start=(t == 0),
                        stop=(t == NT - 1),
                    )

            # ---- assemble raw s2 (still 0.5-scaled); squash host-side ----
            s2 = small.tile([BC, JE], F32, tag="s2")
            nc.vector.tensor_copy(s2[:, 0:E], ps2[:, 0:E])
            nc.vector.tensor_sub(s2[:, E:JE], S[:, E:JE], ps2[:, E:JE])
            nc.sync.dma_start(vout[:], s2[:])

    nc.compile()
    return nc


def _prep_host(x, W):
    bf = ml_dtypes.bfloat16
    # d-major: Wfd[(d,n), je] = W[j, n, e, d]
    Wfd = np.ascontiguousarray(
        np.transpose(W, (3, 1, 0, 2)).reshape(M, JE))
    # wf feed carries the 0.5 iter-1 coupling coefficient
    wf_feed = np.ascontiguousarray(
        (0.5 * Wfd).reshape(NT, 128, JE).transpose(1, 0, 2).reshape(128, FREE)
    ).astype(bf)
    # wft feed: [32 je, m] transposed layout for the 32-contraction g matmuls
    wft_feed = np.ascontiguousarray(Wfd.T).astype(ml_dtypes.float8_e4m3)

    in_maps = []
    for c in range(NCORES):
        xc = x[c * BC:(c + 1) * BC]                 # [32, 6912, 8]
        xd = np.transpose(xc, (2, 1, 0)).reshape(M, BC)   # [(d,n), b]
        xt_feed = np.ascontiguousarray(
            xd.reshape(NT, 128, BC).transpose(1, 0, 2).reshape(128, FREE)
        ).astype(bf)
        in_maps.append({"xt": xt_feed, "wf": wf_feed, "wft": wft_feed})
    return in_maps


def kernel(x, W):
    global _cached
    x = np.asarray(x, dtype=np.float32)
    W = np.asarray(W, dtype=np.float32)
    if _cached is None:
        _cached = _build_program()
    nc = _cached
    in_maps = _prep_host(x, W)
    res = run_bass_kernel_spmd(nc, in_maps, list(range(NCORES)))
    s2 = np.concatenate(
        [res.results[c]["vout"].reshape(BC, J, E) for c in range(NCORES)],
        axis=0,
    ).astype(np.float64)
    s2 *= 2.0   # undo the 0.5 folded into the wf feed
    n2 = np.sum(s2 * s2, axis=-1, keepdims=True)
    v = (n2 / (1.0 + n2)) * s2 / np.sqrt(n2 + EPS)
    return v.astype(np.float32)


if __name__ == "__main__":
    import sys
    sys.path.insert(0, "/root/problem")
    import reference as ref
    inputs = ref.setup_inputs()
    expected = np.asarray(ref.reference(**inputs))
    actual = kernel(np.asarray(inputs["x"]), np.asarray(inputs["W"]))
    err = np.abs(actual - expected)
    scale = np.abs(expected).max()
    print("absmax err:", err.max(), "scale:", scale, "rel:", err.max() / scale)


# revision 4
# speedup vs baseline: 1.0579x; 1.0579x over previous
"""DigitCapsuleLayer forward (2 routing iterations) on 8 Trainium2 cores — V2.1.

d-major restructure: m-rows ordered m = d*N + n so the routing logits
Delta[n,b] dedupe 8x for the sigmoid and the d-fold is cheap DVE adds.

Per core (32 batches):
  phase1:  S[b,je]  = sum_m x[m,b]*Wf[m,je]     (432 PE matmuls; Wf pre-scaled
                                                 by 0.5 host-side, out rescaled)
  squash:  v1 = squash(S'); vtil = [v1_0, -v1_1]; vtT [32je, 32b]
  g:       psg-tile[m128, b32] = WfT-tile^T @ vtT   (432 matmuls, 32-contract)
  T1:      tch_d = psg_d * x_d    (Pool direct / Act copy + DVE mul per d)
  T2:      Delta = sum_d tch_d    (DVE adds, tree, interleaved)
  sigma:   cbf = sigmoid(Delta)   (Act, [128,1728] once)
  T4:      ybf_d = cbf * x_d      (DVE/Pool, overwrites tch)
  s2:      A[b,je] = sum_m ybf[m,b]*Wf[m,je]    (432 PE matmuls)
  out:     s2_j0 = A_j0 ; s2_j1 = S_j1 - A_j1 ; (x2 rescale + squash host-side)
"""

import os
os.environ.setdefault("NEURON_RT_RESET_CORES", "1")

import numpy as np
import ml_dtypes

import concourse.bacc as bacc
import concourse.mybir as mybir
import concourse.tile as tile
from concourse.bass_utils import run_bass_kernel_spmd

B = 256
NCORES = 8
BC = B // NCORES          # 32 batch per core
N = 6912
D = 8
E = 16
J = 2
JE = J * E                # 32
NBT = N // 128            # 54 tiles per d-slice
NT = D * NBT              # 432 tiles total
M = NT * 128              # 55296
HT = 27                   # tiles per half-d-slice
HSL = HT * BC             # 864 free per half-d-slice
SLICE = NBT * BC          # 1728 free per d-slice
FREE = NT * BC            # 13824
EPS = 1e-9

BF16 = mybir.dt.bfloat16
F8 = mybir.dt.float8e4
F32 = mybir.dt.float32

POOL_DIRECT = ()             # (GPSIMD cannot access PSUM on HW)
POOL_T4 = (4, 7)             # d-slices whose T4 mul runs on GpSimd

_cached = None


def _build_program():
    nc = bacc.Bacc("TRN2", num_devices=NCORES)

    xt = nc.dram_tensor("xt", [128, FREE], BF16, kind="ExternalInput")
    wf = nc.dram_tensor("wf", [128, FREE], BF16, kind="ExternalInput")
    wft = nc.dram_tensor("wft", [32, M], F8, kind="ExternalInput")
    vout = nc.dram_tensor("vout", [BC, JE], F32, kind="ExternalOutput")

    with tile.TileContext(nc) as tc:
        with (
            tc.tile_pool(name="big", bufs=1) as big,
            tc.tile_pool(name="small", bufs=1) as small,
            tc.tile_pool(name="tchp", bufs=1) as tchp,
            tc.tile_pool(name="ps_acc", bufs=1, space="PSUM") as ps_acc,
            tc.tile_pool(name="ps_g", bufs=3, space="PSUM") as ps_g,
        ):
            XT = big.tile([128, FREE], BF16, tag="XT")
            WF = big.tile([128, FREE], BF16, tag="WF")
            WFT = big.tile([32, M], F8, tag="WFT")

            # ---- warm the Sqrt act table early ----
            warm = small.tile([BC, J], F32, tag="warm")
            nc.vector.memset(warm[:], 0.0)
            nc.scalar.activation(warm[:], warm[:],
                                 mybir.ActivationFunctionType.Sqrt)

            # ---- DMA: xt+wf interleaved by d-slice, then wft ----
            for d in range(D):
                lo, hi = d * SLICE, (d + 1) * SLICE
                nc.sync.dma_start(XT[:, lo:hi], xt[:, lo:hi])
                nc.sync.dma_start(WF[:, lo:hi], wf[:, lo:hi])
            for i in range(D):
                nc.sync.dma_start(WFT[:, i * N:(i + 1) * N],
                                  wft[:, i * N:(i + 1) * N])

            # ---- phase 1: S = x^T Wf', accumulate over all 432 tiles ----
            ps1 = ps_acc.tile([BC, JE], F32, tag="psacc")
            for t in range(NT):
                nc.tensor.matmul(
                    ps1[:],
                    lhsT=XT[:, t * BC:(t + 1) * BC],
                    rhs=WF[:, t * JE:(t + 1) * JE],
                    start=(t == 0),
                    stop=(t == NT - 1),
                )

            # ---- squash(S') -> v1 -> vtil -> vtT  (Wf carries the 0.5) ----
            S = small.tile([BC, JE], F32, tag="S")
            sq = small.tile([BC, JE], F32, tag="sq")
            n2 = small.tile([BC, J], F32, tag="n2")
            d1 = small.tile([BC, J], F32, tag="d1")
            q = small.tile([BC, J], F32, tag="q")
            p2 = small.tile([BC, J], F32, tag="p2")
            r = small.tile([BC, J], F32, tag="r")
            f = small.tile([BC, J], F32, tag="f")
            vt = small.tile([BC, JE], BF16, tag="vt")
            vtT = small.tile([BC, JE], BF16, tag="vtT")

            # critical chain kept short: everything reads ps1/PSUM directly;
            # the S copy (needed only for the final s2 assembly) is off-chain.
            nc.vector.tensor_copy(S[:], ps1[:])
            nc.vector.tensor_mul(sq[:], S[:], S[:])
            nc.vector.reduce_sum(
                n2[:], sq.rearrange("p (j e) -> p j e", e=E),
                axis=mybir.AxisListType.X,
            )
            # f = +-n2 / ((1+n2) * sqrt(n2+eps)); sign folded into nn
            nn = small.tile([BC, J], F32, tag="nn")
            nc.vector.tensor_scalar_mul(nn[:, 0:1], n2[:, 0:1], 1.0)
            nc.vector.tensor_scalar_mul(nn[:, 1:2], n2[:, 1:2], -1.0)
            nc.vector.tensor_scalar_add(d1[:], n2[:], 1.0)
            nc.vector.tensor_scalar_add(q[:], n2[:], EPS)
            nc.vector.tensor_mul(p2[:], d1[:], d1[:])
            nc.vector.tensor_mul(p2[:], p2[:], q[:])
            nc.vector.reciprocal(r[:], p2[:])
            nc.scalar.activation(r[:], r[:], mybir.ActivationFunctionType.Sqrt)
            nc.vector.tensor_mul(f[:], nn[:], r[:])
            nc.vector.tensor_scalar_mul(vt[:, 0:E], ps1[:, 0:E], f[:, 0:1])
            nc.vector.tensor_scalar_mul(vt[:, E:JE], ps1[:, E:JE], f[:, 1:2])
            nc.vector.transpose(vtT[:], vt[:])
            # pre-load the Sigmoid act table now (depends on vtT so the
            # scheduler cannot float it ahead of the squash Sqrt)
            warm2 = small.tile([BC, JE], F32, tag="warm2")
            nc.scalar.activation(warm2[:], vtT[:],
                                 mybir.ActivationFunctionType.Sigmoid)

            # ---- routing: per half-d-slice g -> T1; T2 interleaved ----
            tch_l = {}

            def g_half(d, h):
                """g matmuls for half-slice (d,h) -> psum tile [128, 864]."""
                psg = ps_g.tile([128, HSL], F32, tag="psg")
                for i in range(HT):
                    t = d * NBT + h * HT + i
                    nc.tensor.matmul(
                        psg[:, i * BC:(i + 1) * BC],
                        lhsT=WFT[:, t * 128:(t + 1) * 128],
                        rhs=vtT[:],
                        start=True, stop=True,
                    )
                return psg

            def t1(d):
                lo = d * SLICE
                tch = tchp.tile([128, SLICE], BF16, tag=f"tch{d}")
                if d in POOL_DIRECT:
                    for h in range(2):
                        psg = g_half(d, h)
                        nc.gpsimd.tensor_mul(
                            tch[:, h * HSL:(h + 1) * HSL], psg[:],
                            XT[:, lo + h * HSL:lo + (h + 1) * HSL])
                else:
                    gbf = tchp.tile([128, SLICE], BF16, tag="gbf", bufs=2)
                    for h in range(2):
                        psg = g_half(d, h)
                        nc.scalar.copy(gbf[:, h * HSL:(h + 1) * HSL], psg[:])
                        nc.vector.tensor_mul(
                            tch[:, h * HSL:(h + 1) * HSL],
                            gbf[:, h * HSL:(h + 1) * HSL],
                            XT[:, lo + h * HSL:lo + (h + 1) * HSL])
                tch_l[d] = tch

            def t2_add(da, db, eng=None):
                for h in range(2):
                    sl = slice(h * HSL, (h + 1) * HSL)
                    nc.vector.tensor_add(tch_l[da][:, sl], tch_l[da][:, sl],
                                         tch_l[db][:, sl])

            # interleave T1 and the T2 tree so adds run as pairs finish
            t1(0); t1(1)
            t2_add(0, 1)
            t1(2); t1(3)
            t2_add(2, 3)
            t1(4); t1(5)
            t2_add(4, 5); t2_add(0, 2)
            t1(6); t1(7)
            t2_add(6, 7); t2_add(4, 6); t2_add(0, 4)
            Delta = tch_l[0]

            # ---- sigmoid on distinct n only ----
            cbf = tchp.tile([128, SLICE], BF16, tag="cbf")
            nc.scalar.activation(cbf[:, 0:HSL], Delta[:, 0:HSL],
                                 mybir.ActivationFunctionType.Sigmoid)
            nc.scalar.activation(cbf[:, HSL:SLICE], Delta[:, HSL:SLICE],
                                 mybir.ActivationFunctionType.Sigmoid)

            # ---- T4 + s2: ybf_d = cbf * x_d ; A += ybf_d^T Wf_d ----
            ps2 = ps_acc.tile([BC, JE], F32, tag="psacc")

            for d in range(D):
                lo = d * SLICE
                ybf = tch_l[d]  # reuse; Delta (d=0) is dead after sigmoid
                eng = nc.gpsimd if d in POOL_T4 else nc.vector
                for h in range(2):
                    eng.tensor_mul(ybf[:, h * HSL:(h + 1) * HSL],
                                   cbf[:, h * HSL:(h + 1) * HSL],
                                   XT[:, lo + h * HSL:lo + (h + 1) * HSL])
                for nb in range(NBT):
                    t = d * NBT + nb
                    nc.tensor.matmul(
                        ps2[:],
                        lhsT=ybf[:, nb * BC:(nb + 1) * BC],
                        rhs=WF[:, t * JE:(t + 1) * JE],
                        start=(t == 0),
                        stop=(t == NT - 1),
                    )

            # ---- assemble raw s2 (still 0.5-scaled); squash host-side ----
            s2 = small.tile([BC, JE], F32, tag="s2")
            nc.vector.tensor_copy(s2[:, 0:E], ps2[:, 0:E])
            nc.vector.tensor_sub(s2[:, E:JE], S[:, E:JE], ps2[:, E:JE])
            nc.sync.dma_start(vout[:], s2[:])

    nc.compile()
    return nc


def _prep_host(x, W):
    bf = ml_dtypes.bfloat16
    # d-major: Wfd[(d,n), je] = W[j, n, e, d]
    Wfd = np.ascontiguousarray(
        np.transpose(W, (3, 1, 0, 2)).reshape(M, JE))
    # wf feed carries the 0.5 iter-1 coupling coefficient
    wf_feed = np.ascontiguousarray(
        (0.5 * Wfd).reshape(NT, 128, JE).transpose(1, 0, 2).reshape(128, FREE)
    ).astype(bf)
    # wft feed: [32 je, m] transposed layout for the 32-contraction g matmuls
    wft_feed = np.ascontiguousarray(Wfd.T).astype(ml_dtypes.float8_e4m3)

    in_maps = []
    for c in range(NCORES):
        xc = x[c * BC:(c + 1) * BC]                 # [32, 6912, 8]
        xd = np.transpose(xc, (2, 1, 0)).reshape(M, BC)   # [(d,n), b]
        xt_feed = np.ascontiguousarray(
            xd.reshape(NT, 128, BC).transpose(1, 0, 2).reshape(128, FREE)
        ).astype(bf)
        in_maps.append({"xt": xt_feed, "wf": wf_feed, "wft": wft_feed})
    return in_maps


def kernel(x, W):
    global _cached
    x = np.asarray(x, dtype=np.float32)
    W = np.asarray(W, dtype=np.float32)
    if _cached is None:
        _cached = _build_program()
    nc = _cached
    in_maps = _prep_host(x, W)
    res = run_bass_kernel_spmd(nc, in_maps, list(range(NCORES)))
    s2 = np.concatenate(
        [res.results[c]["vout"].reshape(BC, J, E) for c in range(NCORES)],
        axis=0,
    ).astype(np.float64)
    s2 *= 2.0   # undo the 0.5 folded into the wf feed
    n2 = np.sum(s2 * s2, axis=-1, keepdims=True)
    v = (n2 / (1.0 + n2)) * s2 / np.sqrt(n2 + EPS)
    return v.astype(np.float32)


if __name__ == "__main__":
    import sys
    sys.path.insert(0, "/root/problem")
    import reference as ref
    inputs = ref.setup_inputs()
    expected = np.asarray(ref.reference(**inputs))
    actual = kernel(np.asarray(inputs["x"]), np.asarray(inputs["W"]))
    err = np.abs(actual - expected)
    scale = np.abs(expected).max()
    print("absmax err:", err.max(), "scale:", scale, "rel:", err.max() / scale)


# revision 5
# speedup vs baseline: 1.0605x; 1.0025x over previous
"""DigitCapsuleLayer forward (2 routing iterations) on 8 Trainium2 cores — V2.1.

d-major restructure: m-rows ordered m = d*N + n so the routing logits
Delta[n,b] dedupe 8x for the sigmoid and the d-fold is cheap DVE adds.

Per core (32 batches):
  phase1:  S[b,je]  = sum_m x[m,b]*Wf[m,je]     (432 PE matmuls; Wf pre-scaled
                                                 by 0.5 host-side, out rescaled)
  squash:  v1 = squash(S'); vtil = [v1_0, -v1_1]; vtT [32je, 32b]
  g:       psg-tile[m128, b32] = WfT-tile^T @ vtT   (432 matmuls, 32-contract)
  T1:      tch_d = psg_d * x_d    (Pool direct / Act copy + DVE mul per d)
  T2:      Delta = sum_d tch_d    (DVE adds, tree, interleaved)
  sigma:   cbf = sigmoid(Delta)   (Act, [128,1728] once)
  T4:      ybf_d = cbf * x_d      (DVE/Pool, overwrites tch)
  s2:      A[b,je] = sum_m ybf[m,b]*Wf[m,je]    (432 PE matmuls)
  out:     s2_j0 = A_j0 ; s2_j1 = S_j1 - A_j1 ; (x2 rescale + squash host-side)
"""

import os
os.environ.setdefault("NEURON_RT_RESET_CORES", "1")

import numpy as np
import ml_dtypes

import concourse.bacc as bacc
import concourse.mybir as mybir
import concourse.tile as tile
from concourse.bass_utils import run_bass_kernel_spmd

B = 256
NCORES = 8
BC = B // NCORES          # 32 batch per core
N = 6912
D = 8
E = 16
J = 2
JE = J * E                # 32
NBT = N // 128            # 54 tiles per d-slice
NT = D * NBT              # 432 tiles total
M = NT * 128              # 55296
HT = 27                   # tiles per half-d-slice
HSL = HT * BC             # 864 free per half-d-slice
SLICE = NBT * BC          # 1728 free per d-slice
FREE = NT * BC            # 13824
EPS = 1e-9

BF16 = mybir.dt.bfloat16
F8 = mybir.dt.float8e4
F32 = mybir.dt.float32

POOL_DIRECT = ()             # (GPSIMD cannot access PSUM on HW)
POOL_T4 = (4, 7)             # d-slices whose T4 mul runs on GpSimd

_cached = None


def _build_program():
    nc = bacc.Bacc("TRN2", num_devices=NCORES)

    xt = nc.dram_tensor("xt", [128, FREE], BF16, kind="ExternalInput")
    wf = nc.dram_tensor("wf", [128, FREE], BF16, kind="ExternalInput")
    wft = nc.dram_tensor("wft", [32, M], F8, kind="ExternalInput")
    vout = nc.dram_tensor("vout", [BC, JE], F32, kind="ExternalOutput")

    with tile.TileContext(nc) as tc:
        with (
            tc.tile_pool(name="big", bufs=1) as big,
            tc.tile_pool(name="small", bufs=1) as small,
            tc.tile_pool(name="tchp", bufs=1) as tchp,
            tc.tile_pool(name="ps_acc", bufs=1, space="PSUM") as ps_acc,
            tc.tile_pool(name="ps_g", bufs=3, space="PSUM") as ps_g,
        ):
            XT = big.tile([128, FREE], BF16, tag="XT")
            WF = big.tile([128, FREE], BF16, tag="WF")
            WFT = big.tile([32, M], F8, tag="WFT")

            # ---- warm the Sqrt act table early ----
            warm = small.tile([BC, J], F32, tag="warm")
            nc.vector.memset(warm[:], 0.0)
            nc.scalar.activation(warm[:], warm[:],
                                 mybir.ActivationFunctionType.Sqrt)

            # ---- DMA: xt+wf interleaved by d-slice, then wft ----
            for d in range(D):
                lo, hi = d * SLICE, (d + 1) * SLICE
                nc.sync.dma_start(XT[:, lo:hi], xt[:, lo:hi])
                nc.sync.dma_start(WF[:, lo:hi], wf[:, lo:hi])
            for i in range(D):
                nc.sync.dma_start(WFT[:, i * N:(i + 1) * N],
                                  wft[:, i * N:(i + 1) * N])

            # ---- phase 1: S = x^T Wf', accumulate over all 432 tiles ----
            ps1 = ps_acc.tile([BC, JE], F32, tag="psacc")
            for t in range(NT):
                nc.tensor.matmul(
                    ps1[:],
                    lhsT=XT[:, t * BC:(t + 1) * BC],
                    rhs=WF[:, t * JE:(t + 1) * JE],
                    start=(t == 0),
                    stop=(t == NT - 1),
                )

            # ---- squash(S') -> v1 -> vtil -> vtT  (Wf carries the 0.5) ----
            S = small.tile([BC, JE], F32, tag="S")
            sq = small.tile([BC, JE], F32, tag="sq")
            n2 = small.tile([BC, J], F32, tag="n2")
            d1 = small.tile([BC, J], F32, tag="d1")
            q = small.tile([BC, J], F32, tag="q")
            p2 = small.tile([BC, J], F32, tag="p2")
            r = small.tile([BC, J], F32, tag="r")
            f = small.tile([BC, J], F32, tag="f")
            vt = small.tile([BC, JE], BF16, tag="vt")
            vtT = small.tile([BC, JE], BF16, tag="vtT")

            # critical chain kept short: everything reads ps1/PSUM directly;
            # the S copy (needed only for the final s2 assembly) is off-chain.
            nc.vector.tensor_copy(S[:], ps1[:])
            nc.vector.tensor_mul(sq[:], S[:], S[:])
            nc.vector.reduce_sum(
                n2[:], sq.rearrange("p (j e) -> p j e", e=E),
                axis=mybir.AxisListType.X,
            )
            # f = +-n2 / ((1+n2) * sqrt(n2+eps)); sign folded into nn
            nn = small.tile([BC, J], F32, tag="nn")
            nc.vector.tensor_scalar_mul(nn[:, 0:1], n2[:, 0:1], 1.0)
            nc.vector.tensor_scalar_mul(nn[:, 1:2], n2[:, 1:2], -1.0)
            nc.vector.tensor_scalar_add(d1[:], n2[:], 1.0)
            nc.vector.tensor_scalar_add(q[:], n2[:], EPS)
            nc.vector.tensor_mul(p2[:], d1[:], d1[:])
            nc.vector.tensor_mul(p2[:], p2[:], q[:])
            nc.vector.reciprocal(r[:], p2[:])
            nc.scalar.activation(r[:], r[:], mybir.ActivationFunctionType.Sqrt)
            nc.vector.tensor_mul(f[:], nn[:], r[:])
            nc.vector.tensor_scalar_mul(vt[:, 0:E], ps1[:, 0:E], f[:, 0:1])
            nc.vector.tensor_scalar_mul(vt[:, E:JE], ps1[:, E:JE], f[:, 1:2])
            nc.vector.transpose(vtT[:], vt[:])
            # pre-load the Sigmoid act table now (depends on vtT so the
            # scheduler cannot float it ahead of the squash Sqrt)
            warm2 = small.tile([BC, JE], F32, tag="warm2")
            nc.scalar.activation(warm2[:], vtT[:],
                                 mybir.ActivationFunctionType.Sigmoid)

            # ---- routing: per half-d-slice g -> T1; T2 interleaved ----
            tch_l = {}

            def g_half(d, h):
                """g matmuls for half-slice (d,h) -> psum tile [128, 864]."""
                psg = ps_g.tile([128, HSL], F32, tag="psg")
                for i in range(HT):
                    t = d * NBT + h * HT + i
                    nc.tensor.matmul(
                        psg[:, i * BC:(i + 1) * BC],
                        lhsT=WFT[:, t * 128:(t + 1) * 128],
                        rhs=vtT[:],
                        start=True, stop=True,
                    )
                return psg

            def t1(d):
                lo = d * SLICE
                tch = tchp.tile([128, SLICE], BF16, tag=f"tch{d}")
                if d in POOL_DIRECT:
                    for h in range(2):
                        psg = g_half(d, h)
                        nc.gpsimd.tensor_mul(
                            tch[:, h * HSL:(h + 1) * HSL], psg[:],
                            XT[:, lo + h * HSL:lo + (h + 1) * HSL])
                else:
                    gbf = tchp.tile([128, SLICE], BF16, tag="gbf", bufs=2)
                    for h in range(2):
                        psg = g_half(d, h)
                        nc.scalar.copy(gbf[:, h * HSL:(h + 1) * HSL], psg[:])
                        nc.vector.tensor_mul(
                            tch[:, h * HSL:(h + 1) * HSL],
                            gbf[:, h * HSL:(h + 1) * HSL],
                            XT[:, lo + h * HSL:lo + (h + 1) * HSL])
                tch_l[d] = tch

            def t2_add(da, db, eng=None):
                for h in range(2):
                    sl = slice(h * HSL, (h + 1) * HSL)
                    e = eng if eng is not None else nc.vector
                    e.tensor_add(tch_l[da][:, sl], tch_l[da][:, sl],
                                 tch_l[db][:, sl])

            # interleave T1 and the T2 tree so adds run as pairs finish
            t1(0); t1(1)
            t2_add(0, 1)
            t1(2); t1(3)
            t2_add(2, 3)
            t1(4); t1(5)
            t2_add(4, 5, nc.gpsimd); t2_add(0, 2)
            t1(6); t1(7)
            t2_add(6, 7); t2_add(4, 6); t2_add(0, 4)
            Delta = tch_l[0]

            # ---- sigmoid on distinct n only ----
            cbf = tchp.tile([128, SLICE], BF16, tag="cbf")
            nc.scalar.activation(cbf[:, 0:HSL], Delta[:, 0:HSL],
                                 mybir.ActivationFunctionType.Sigmoid)
            nc.scalar.activation(cbf[:, HSL:SLICE], Delta[:, HSL:SLICE],
                                 mybir.ActivationFunctionType.Sigmoid)

            # ---- T4 + s2: ybf_d = cbf * x_d ; A += ybf_d^T Wf_d ----
            ps2 = ps_acc.tile([BC, JE], F32, tag="psacc")

            for d in range(D):
                lo = d * SLICE
                ybf = tch_l[d]  # reuse; Delta (d=0) is dead after sigmoid
                eng = nc.gpsimd if d in POOL_T4 else nc.vector
                for h in range(2):
                    eng.tensor_mul(ybf[:, h * HSL:(h + 1) * HSL],
                                   cbf[:, h * HSL:(h + 1) * HSL],
                                   XT[:, lo + h * HSL:lo + (h + 1) * HSL])
                for nb in range(NBT):
                    t = d * NBT + nb
                    nc.tensor.matmul(
                        ps2[:],
                        lhsT=ybf[:, nb * BC:(nb + 1) * BC],
                        rhs=WF[:, t * JE:(t + 1) * JE],
                        start=(t == 0),
                        stop=(t == NT - 1),
                    )

            # ---- assemble raw s2 (still 0.5-scaled); squash host-side ----
            s2 = small.tile([BC, JE], F32, tag="s2")
            nc.vector.tensor_copy(s2[:, 0:E], ps2[:, 0:E])
            nc.vector.tensor_sub(s2[:, E:JE], S[:, E:JE], ps2[:, E:JE])
            nc.sync.dma_start(vout[:], s2[:])

    nc.compile()
    return nc


def _prep_host(x, W):
    bf = ml_dtypes.bfloat16
    # d-major: Wfd[(d,n), je] = W[j, n, e, d]
    Wfd = np.ascontiguousarray(
        np.transpose(W, (3, 1, 0, 2)).reshape(M, JE))
    # wf feed carries the 0.5 iter-1 coupling coefficient
    wf_feed = np.ascontiguousarray(
        (0.5 * Wfd).reshape(NT, 128, JE).transpose(1, 0, 2).reshape(128, FREE)
    ).astype(bf)
    # wft feed: [32 je, m] transposed layout for the 32-contraction g matmuls
    wft_feed = np.ascontiguousarray(Wfd.T).astype(ml_dtypes.float8_e4m3)

    in_maps = []
    for c in range(NCORES):
        xc = x[c * BC:(c + 1) * BC]                 # [32, 6912, 8]
        xd = np.transpose(xc, (2, 1, 0)).reshape(M, BC)   # [(d,n), b]
        xt_feed = np.ascontiguousarray(
            xd.reshape(NT, 128, BC).transpose(1, 0, 2).reshape(128, FREE)
        ).astype(bf)
        in_maps.append({"xt": xt_feed, "wf": wf_feed, "wft": wft_feed})
    return in_maps


def kernel(x, W):
    global _cached
    x = np.asarray(x, dtype=np.float32)
    W = np.asarray(W, dtype=np.float32)
    if _cached is None:
        _cached = _build_program()
    nc = _cached
    in_maps = _prep_host(x, W)
    res = run_bass_kernel_spmd(nc, in_maps, list(range(NCORES)))
    s2 = np.concatenate(
        [res.results[c]["vout"].reshape(BC, J, E) for c in range(NCORES)],
        axis=0,
    ).astype(np.float64)
    s2 *= 2.0   # undo the 0.5 folded into the wf feed
    n2 = np.sum(s2 * s2, axis=-1, keepdims=True)
    v = (n2 / (1.0 + n2)) * s2 / np.sqrt(n2 + EPS)
    return v.astype(np.float32)


if __name__ == "__main__":
    import sys
    sys.path.insert(0, "/root/problem")
    import reference as ref
    inputs = ref.setup_inputs()
    expected = np.asarray(ref.reference(**inputs))
    actual = kernel(np.asarray(inputs["x"]), np.asarray(inputs["W"]))
    err = np.abs(actual - expected)
    scale = np.abs(expected).max()
    print("absmax err:", err.max(), "scale:", scale, "rel:", err.max() / scale)


# revision 6
# speedup vs baseline: 1.0735x; 1.0123x over previous
"""DigitCapsuleLayer forward (2 routing iterations) on 8 Trainium2 cores — V2.1.

d-major restructure: m-rows ordered m = d*N + n so the routing logits
Delta[n,b] dedupe 8x for the sigmoid and the d-fold is cheap DVE adds.

Per core (32 batches):
  phase1:  S[b,je]  = sum_m x[m,b]*Wf[m,je]     (432 PE matmuls; Wf pre-scaled
                                                 by 0.5 host-side, out rescaled)
  squash:  v1 = squash(S'); vtil = [v1_0, -v1_1]; vtT [32je, 32b]
  g:       psg-tile[m128, b32] = WfT-tile^T @ vtT   (432 matmuls, 32-contract)
  T1:      tch_d = psg_d * x_d    (Pool direct / Act copy + DVE mul per d)
  T2:      Delta = sum_d tch_d    (DVE adds, tree, interleaved)
  sigma:   cbf = sigmoid(Delta)   (Act, [128,1728] once)
  T4:      ybf_d = cbf * x_d      (DVE/Pool, overwrites tch)
  s2:      A[b,je] = sum_m ybf[m,b]*Wf[m,je]    (432 PE matmuls)
  out:     s2_j0 = A_j0 ; s2_j1 = S_j1 - A_j1 ; (x2 rescale + squash host-side)
"""

import os
os.environ.setdefault("NEURON_RT_RESET_CORES", "1")

import numpy as np
import ml_dtypes

import concourse.bacc as bacc
import concourse.mybir as mybir
import concourse.tile as tile
from concourse.bass_utils import run_bass_kernel_spmd

B = 256
NCORES = 8
BC = B // NCORES          # 32 batch per core
N = 6912
D = 8
E = 16
J = 2
JE = J * E                # 32
NBT = N // 128            # 54 tiles per d-slice
NT = D * NBT              # 432 tiles total
M = NT * 128              # 55296
HT = 27                   # tiles per half-d-slice
HSL = HT * BC             # 864 free per half-d-slice
SLICE = NBT * BC          # 1728 free per d-slice
FREE = NT * BC            # 13824
EPS = 1e-9

BF16 = mybir.dt.bfloat16
F8 = mybir.dt.float8e4
F32 = mybir.dt.float32

POOL_DIRECT = ()             # (GPSIMD cannot access PSUM on HW)
POOL_T4H = ((4, 0), (6, 0), (7, 1))  # (d,h) T4 half-muls on GpSimd

_cached = None


def _build_program():
    nc = bacc.Bacc("TRN2", num_devices=NCORES)

    xt = nc.dram_tensor("xt", [128, FREE], BF16, kind="ExternalInput")
    wf = nc.dram_tensor("wf", [128, FREE], BF16, kind="ExternalInput")
    wft = nc.dram_tensor("wft", [32, M], F8, kind="ExternalInput")
    vout = nc.dram_tensor("vout", [BC, JE], F32, kind="ExternalOutput")
    sout = nc.dram_tensor("sout", [BC, JE], F32, kind="ExternalOutput")

    with tile.TileContext(nc) as tc:
        with (
            tc.tile_pool(name="big", bufs=1) as big,
            tc.tile_pool(name="small", bufs=1) as small,
            tc.tile_pool(name="tchp", bufs=1) as tchp,
            tc.tile_pool(name="ps_acc", bufs=1, space="PSUM") as ps_acc,
            tc.tile_pool(name="ps_g", bufs=3, space="PSUM") as ps_g,
        ):
            XT = big.tile([128, FREE], BF16, tag="XT")
            WF = big.tile([128, FREE], BF16, tag="WF")
            WFT = big.tile([32, M], F8, tag="WFT")

            # ---- warm the Sqrt act table early ----
            warm = small.tile([BC, J], F32, tag="warm")
            nc.vector.memset(warm[:], 0.0)
            nc.scalar.activation(warm[:], warm[:],
                                 mybir.ActivationFunctionType.Sqrt)

            # ---- DMA: xt+wf interleaved by d-slice, then wft ----
            for d in range(D):
                lo, hi = d * SLICE, (d + 1) * SLICE
                if d < 6:
                    nc.sync.dma_start(XT[:, lo:hi], xt[:, lo:hi])
                    nc.sync.dma_start(WF[:, lo:hi], wf[:, lo:hi])
                else:
                    mid = lo + HSL
                    nc.sync.dma_start(XT[:, lo:mid], xt[:, lo:mid])
                    nc.sync.dma_start(WF[:, lo:mid], wf[:, lo:mid])
                    nc.sync.dma_start(XT[:, mid:hi], xt[:, mid:hi])
                    nc.sync.dma_start(WF[:, mid:hi], wf[:, mid:hi])
            for i in range(D):
                nc.sync.dma_start(WFT[:, i * N:(i + 1) * N],
                                  wft[:, i * N:(i + 1) * N])

            # ---- phase 1: S = x^T Wf', accumulate over all 432 tiles ----
            ps1 = ps_acc.tile([BC, JE], F32, tag="psacc")
            for t in range(NT):
                nc.tensor.matmul(
                    ps1[:],
                    lhsT=XT[:, t * BC:(t + 1) * BC],
                    rhs=WF[:, t * JE:(t + 1) * JE],
                    start=(t == 0),
                    stop=(t == NT - 1),
                )

            # ---- squash(S') -> v1 -> vtil -> vtT  (Wf carries the 0.5) ----
            S = small.tile([BC, JE], F32, tag="S")
            sq = small.tile([BC, JE], F32, tag="sq")
            n2 = small.tile([BC, J], F32, tag="n2")
            d1 = small.tile([BC, J], F32, tag="d1")
            q = small.tile([BC, J], F32, tag="q")
            p2 = small.tile([BC, J], F32, tag="p2")
            r = small.tile([BC, J], F32, tag="r")
            f = small.tile([BC, J], F32, tag="f")
            vt = small.tile([BC, JE], BF16, tag="vt")
            vtT = small.tile([BC, JE], BF16, tag="vtT")

            # critical chain kept short: everything reads ps1/PSUM directly;
            # the S copy (needed only for the final s2 assembly) is off-chain.
            nc.vector.tensor_copy(S[:], ps1[:])
            nc.vector.tensor_mul(sq[:], S[:], S[:])
            nc.vector.reduce_sum(
                n2[:], sq.rearrange("p (j e) -> p j e", e=E),
                axis=mybir.AxisListType.X,
            )
            # f = +-n2 / ((1+n2) * sqrt(n2+eps)); sign folded into nn
            nn = small.tile([BC, J], F32, tag="nn")
            nc.vector.tensor_scalar_mul(nn[:, 0:1], n2[:, 0:1], 1.0)
            nc.vector.tensor_scalar_mul(nn[:, 1:2], n2[:, 1:2], -1.0)
            nc.vector.tensor_scalar_add(d1[:], n2[:], 1.0)
            nc.vector.tensor_scalar_add(q[:], n2[:], EPS)
            nc.vector.tensor_mul(p2[:], d1[:], d1[:])
            nc.vector.tensor_mul(p2[:], p2[:], q[:])
            nc.vector.reciprocal(r[:], p2[:])
            nc.scalar.activation(r[:], r[:], mybir.ActivationFunctionType.Sqrt)
            nc.vector.tensor_mul(f[:], nn[:], r[:])
            nc.vector.tensor_scalar_mul(vt[:, 0:E], ps1[:, 0:E], f[:, 0:1])
            nc.vector.tensor_scalar_mul(vt[:, E:JE], ps1[:, E:JE], f[:, 1:2])
            nc.vector.transpose(vtT[:], vt[:])


            # ---- routing: per half-d-slice g -> T1; T2 interleaved ----
            tch_l = {}

            def g_half(d, h):
                """g matmuls for half-slice (d,h) -> psum tile [128, 864]."""
                psg = ps_g.tile([128, HSL], F32, tag="psg")
                for i in range(HT):
                    t = d * NBT + h * HT + i
                    nc.tensor.matmul(
                        psg[:, i * BC:(i + 1) * BC],
                        lhsT=WFT[:, t * 128:(t + 1) * 128],
                        rhs=vtT[:],
                        start=True, stop=True,
                    )
                return psg

            def t1(d):
                lo = d * SLICE
                tch = tchp.tile([128, SLICE], BF16, tag=f"tch{d}")
                if d in POOL_DIRECT:
                    for h in range(2):
                        psg = g_half(d, h)
                        nc.gpsimd.tensor_mul(
                            tch[:, h * HSL:(h + 1) * HSL], psg[:],
                            XT[:, lo + h * HSL:lo + (h + 1) * HSL])
                else:
                    gbf = tchp.tile([128, SLICE], BF16, tag="gbf", bufs=2)
                    for h in range(2):
                        psg = g_half(d, h)
                        nc.scalar.copy(gbf[:, h * HSL:(h + 1) * HSL], psg[:])
                        nc.vector.tensor_mul(
                            tch[:, h * HSL:(h + 1) * HSL],
                            gbf[:, h * HSL:(h + 1) * HSL],
                            XT[:, lo + h * HSL:lo + (h + 1) * HSL])
                tch_l[d] = tch

            def t2_add(da, db, eng=None):
                for h in range(2):
                    sl = slice(h * HSL, (h + 1) * HSL)
                    e = eng if eng is not None else nc.vector
                    e.tensor_add(tch_l[da][:, sl], tch_l[da][:, sl],
                                 tch_l[db][:, sl])

            # interleave T1 and the T2 tree so adds run as pairs finish
            t1(0); t1(1)
            # pre-load the Sigmoid act table in an Act gap (Copy runs in any
            # set, so copies before/after are unaffected)
            warm2 = small.tile([BC, JE], F32, tag="warm2")
            nc.scalar.activation(warm2[:], vtT[:],
                                 mybir.ActivationFunctionType.Sigmoid)
            t2_add(0, 1)
            t1(2); t1(3)
            t2_add(2, 3)
            t1(4); t1(5)
            t2_add(4, 5, nc.gpsimd); t2_add(0, 2)
            t1(6); t1(7)
            t2_add(6, 7); t2_add(4, 6); t2_add(0, 4)
            Delta = tch_l[0]

            # ---- sigmoid on distinct n only ----
            cbf = tchp.tile([128, SLICE], BF16, tag="cbf")
            nc.scalar.activation(cbf[:, 0:HSL], Delta[:, 0:HSL],
                                 mybir.ActivationFunctionType.Sigmoid)
            nc.scalar.activation(cbf[:, HSL:SLICE], Delta[:, HSL:SLICE],
                                 mybir.ActivationFunctionType.Sigmoid)

            # ---- T4 + s2: ybf_d = cbf * x_d ; A += ybf_d^T Wf_d ----
            ps2 = ps_acc.tile([BC, JE], F32, tag="psacc")

            for d in range(D):
                lo = d * SLICE
                ybf = tch_l[d]  # reuse; Delta (d=0) is dead after sigmoid
                for h in range(2):
                    eng = nc.gpsimd if (d, h) in POOL_T4H else nc.vector
                    eng.tensor_mul(ybf[:, h * HSL:(h + 1) * HSL],
                                   cbf[:, h * HSL:(h + 1) * HSL],
                                   XT[:, lo + h * HSL:lo + (h + 1) * HSL])
                for nb in range(NBT):
                    t = d * NBT + nb
                    nc.tensor.matmul(
                        ps2[:],
                        lhsT=ybf[:, nb * BC:(nb + 1) * BC],
                        rhs=WF[:, t * JE:(t + 1) * JE],
                        start=(t == 0),
                        stop=(t == NT - 1),
                    )

            # ---- ship S early and raw A at the end; assembly host-side ----
            nc.sync.dma_start(sout[:], S[:])
            a2 = small.tile([BC, JE], F32, tag="a2")
            nc.vector.tensor_copy(a2[:], ps2[:])
            nc.sync.dma_start(vout[:], a2[:])

    nc.compile()
    return nc


def _prep_host(x, W):
    bf = ml_dtypes.bfloat16
    # d-major: Wfd[(d,n), je] = W[j, n, e, d]
    Wfd = np.ascontiguousarray(
        np.transpose(W, (3, 1, 0, 2)).reshape(M, JE))
    # wf feed carries the 0.5 iter-1 coupling coefficient
    wf_feed = np.ascontiguousarray(
        (0.5 * Wfd).reshape(NT, 128, JE).transpose(1, 0, 2).reshape(128, FREE)
    ).astype(bf)
    # wft feed: [32 je, m] transposed layout for the 32-contraction g matmuls
    wft_feed = np.ascontiguousarray(Wfd.T).astype(ml_dtypes.float8_e4m3)

    in_maps = []
    for c in range(NCORES):
        xc = x[c * BC:(c + 1) * BC]                 # [32, 6912, 8]
        xd = np.transpose(xc, (2, 1, 0)).reshape(M, BC)   # [(d,n), b]
        xt_feed = np.ascontiguousarray(
            xd.reshape(NT, 128, BC).transpose(1, 0, 2).reshape(128, FREE)
        ).astype(bf)
        in_maps.append({"xt": xt_feed, "wf": wf_feed, "wft": wft_feed})
    return in_maps


def kernel(x, W):
    global _cached
    x = np.asarray(x, dtype=np.float32)
    W = np.asarray(W, dtype=np.float32)
    if _cached is None:
        _cached = _build_program()
    nc = _cached
    in_maps = _prep_host(x, W)
    res = run_bass_kernel_spmd(nc, in_maps, list(range(NCORES)))
    A = np.concatenate(
        [res.results[c]["vout"].reshape(BC, J, E) for c in range(NCORES)],
        axis=0,
    ).astype(np.float64)
    Sf = np.concatenate(
        [res.results[c]["sout"].reshape(BC, J, E) for c in range(NCORES)],
        axis=0,
    ).astype(np.float64)
    s2 = np.empty_like(A)
    s2[:, 0, :] = A[:, 0, :]
    s2[:, 1, :] = Sf[:, 1, :] - A[:, 1, :]
    s2 *= 2.0   # undo the 0.5 folded into the wf feed
    n2 = np.sum(s2 * s2, axis=-1, keepdims=True)
    v = (n2 / (1.0 + n2)) * s2 / np.sqrt(n2 + EPS)
    return v.astype(np.float32)


if __name__ == "__main__":
    import sys
    sys.path.insert(0, "/root/problem")
    import reference as ref
    inputs = ref.setup_inputs()
    expected = np.asarray(ref.reference(**inputs))
    actual = kernel(np.asarray(inputs["x"]), np.asarray(inputs["W"]))
    err = np.abs(actual - expected)
    scale = np.abs(expected).max()
    print("absmax err:", err.max(), "scale:", scale, "rel:", err.max() / scale)


# revision 7
# speedup vs baseline: 1.0743x; 1.0007x over previous
"""DigitCapsuleLayer forward (2 routing iterations) on 8 Trainium2 cores — V2.1.

d-major restructure: m-rows ordered m = d*N + n so the routing logits
Delta[n,b] dedupe 8x for the sigmoid and the d-fold is cheap DVE adds.

Per core (32 batches):
  phase1:  S[b,je]  = sum_m x[m,b]*Wf[m,je]     (432 PE matmuls; Wf pre-scaled
                                                 by 0.5 host-side, out rescaled)
  squash:  v1 = squash(S'); vtil = [v1_0, -v1_1]; vtT [32je, 32b]
  g:       psg-tile[m128, b32] = WfT-tile^T @ vtT   (432 matmuls, 32-contract)
  T1:      tch_d = psg_d * x_d    (Pool direct / Act copy + DVE mul per d)
  T2:      Delta = sum_d tch_d    (DVE adds, tree, interleaved)
  sigma:   cbf = sigmoid(Delta)   (Act, [128,1728] once)
  T4:      ybf_d = cbf * x_d      (DVE/Pool, overwrites tch)
  s2:      A[b,je] = sum_m ybf[m,b]*Wf[m,je]    (432 PE matmuls)
  out:     s2_j0 = A_j0 ; s2_j1 = S_j1 - A_j1 ; (x2 rescale + squash host-side)
"""

import os
os.environ.setdefault("NEURON_RT_RESET_CORES", "1")

import numpy as np
import ml_dtypes

import concourse.bacc as bacc
import concourse.mybir as mybir
import concourse.tile as tile
from concourse.bass_utils import run_bass_kernel_spmd

B = 256
NCORES = 8
BC = B // NCORES          # 32 batch per core
N = 6912
D = 8
E = 16
J = 2
JE = J * E                # 32
NBT = N // 128            # 54 tiles per d-slice
NT = D * NBT              # 432 tiles total
M = NT * 128              # 55296
HT = 27                   # tiles per half-d-slice
HSL = HT * BC             # 864 free per half-d-slice
SLICE = NBT * BC          # 1728 free per d-slice
FREE = NT * BC            # 13824
EPS = 1e-9

BF16 = mybir.dt.bfloat16
F8 = mybir.dt.float8e4
F32 = mybir.dt.float32

POOL_DIRECT = ()             # (GPSIMD cannot access PSUM on HW)
POOL_T4H = ((4, 0), (6, 0), (7, 1))  # (d,h) T4 half-muls on GpSimd

_cached = None


def _build_program():
    nc = bacc.Bacc("TRN2", num_devices=NCORES)

    xt = nc.dram_tensor("xt", [128, FREE], BF16, kind="ExternalInput")
    wf = nc.dram_tensor("wf", [128, FREE], BF16, kind="ExternalInput")
    wft = nc.dram_tensor("wft", [32, M], F8, kind="ExternalInput")
    vout = nc.dram_tensor("vout", [BC, JE], F32, kind="ExternalOutput")
    sout = nc.dram_tensor("sout", [BC, JE], F32, kind="ExternalOutput")

    with tile.TileContext(nc) as tc:
        with (
            tc.tile_pool(name="big", bufs=1) as big,
            tc.tile_pool(name="small", bufs=1) as small,
            tc.tile_pool(name="tchp", bufs=1) as tchp,
            tc.tile_pool(name="ps_acc", bufs=1, space="PSUM") as ps_acc,
            tc.tile_pool(name="ps_g", bufs=3, space="PSUM") as ps_g,
        ):
            XT = big.tile([128, FREE], BF16, tag="XT")
            WF = big.tile([128, FREE], BF16, tag="WF")
            WFT = big.tile([32, M], F8, tag="WFT")

            # ---- warm the Sqrt act table early ----
            warm = small.tile([BC, J], F32, tag="warm")
            nc.vector.memset(warm[:], 0.0)
            nc.scalar.activation(warm[:], warm[:],
                                 mybir.ActivationFunctionType.Sqrt)

            # ---- DMA: xt+wf interleaved by d-slice, then wft ----
            for d in range(D):
                lo, hi = d * SLICE, (d + 1) * SLICE
                if d < 6:
                    nc.sync.dma_start(XT[:, lo:hi], xt[:, lo:hi])
                    nc.sync.dma_start(WF[:, lo:hi], wf[:, lo:hi])
                else:
                    mid = lo + HSL
                    nc.sync.dma_start(XT[:, lo:mid], xt[:, lo:mid])
                    nc.sync.dma_start(WF[:, lo:mid], wf[:, lo:mid])
                    nc.sync.dma_start(XT[:, mid:hi], xt[:, mid:hi])
                    nc.sync.dma_start(WF[:, mid:hi], wf[:, mid:hi])
            for i in range(D):
                nc.sync.dma_start(WFT[:, i * N:(i + 1) * N],
                                  wft[:, i * N:(i + 1) * N])

            # ---- phase 1: S = x^T Wf', accumulate over all 432 tiles ----
            ps1 = ps_acc.tile([BC, JE], F32, tag="psacc")
            for t in range(NT):
                nc.tensor.matmul(
                    ps1[:],
                    lhsT=XT[:, t * BC:(t + 1) * BC],
                    rhs=WF[:, t * JE:(t + 1) * JE],
                    start=(t == 0),
                    stop=(t == NT - 1),
                )

            # ---- squash(S') -> v1 -> vtil -> vtT  (Wf carries the 0.5) ----
            S = small.tile([BC, JE], F32, tag="S")
            sq = small.tile([BC, JE], F32, tag="sq")
            n2 = small.tile([BC, J], F32, tag="n2")
            d1 = small.tile([BC, J], F32, tag="d1")
            q = small.tile([BC, J], F32, tag="q")
            p2 = small.tile([BC, J], F32, tag="p2")
            r = small.tile([BC, J], F32, tag="r")
            f = small.tile([BC, J], F32, tag="f")
            vt = small.tile([BC, JE], BF16, tag="vt")
            vtT = small.tile([BC, JE], BF16, tag="vtT")

            # critical chain kept short: everything reads ps1/PSUM directly;
            # the S copy (needed only for the final s2 assembly) is off-chain.
            nc.vector.tensor_copy(S[:], ps1[:])
            nc.vector.tensor_mul(sq[:], S[:], S[:])
            nc.vector.reduce_sum(
                n2[:], sq.rearrange("p (j e) -> p j e", e=E),
                axis=mybir.AxisListType.X,
            )
            # f = +-n2 / ((1+n2) * sqrt(n2+eps)); sign folded into nn
            nn = small.tile([BC, J], F32, tag="nn")
            nc.vector.tensor_scalar_mul(nn[:, 0:1], n2[:, 0:1], 1.0)
            nc.vector.tensor_scalar_mul(nn[:, 1:2], n2[:, 1:2], -1.0)
            nc.vector.tensor_scalar_add(d1[:], n2[:], 1.0)
            nc.vector.tensor_scalar_add(q[:], n2[:], EPS)
            nc.vector.tensor_mul(p2[:], d1[:], d1[:])
            nc.vector.tensor_mul(p2[:], p2[:], q[:])
            nc.vector.reciprocal(r[:], p2[:])
            nc.scalar.activation(r[:], r[:], mybir.ActivationFunctionType.Sqrt)
            nc.vector.tensor_mul(f[:], nn[:], r[:])
            nc.vector.tensor_scalar_mul(vt[:, 0:E], ps1[:, 0:E], f[:, 0:1])
            nc.vector.tensor_scalar_mul(vt[:, E:JE], ps1[:, E:JE], f[:, 1:2])
            nc.vector.transpose(vtT[:], vt[:])


            # ---- routing: per half-d-slice g -> T1; T2 interleaved ----
            tch_l = {}

            def g_half(d, h):
                """g matmuls for half-slice (d,h) -> psum tile [128, 864]."""
                psg = ps_g.tile([128, HSL], F32, tag="psg")
                for i in range(HT):
                    t = d * NBT + h * HT + i
                    nc.tensor.matmul(
                        psg[:, i * BC:(i + 1) * BC],
                        lhsT=WFT[:, t * 128:(t + 1) * 128],
                        rhs=vtT[:],
                        start=True, stop=True,
                    )
                return psg

            def t1(d):
                lo = d * SLICE
                tch = tchp.tile([128, SLICE], BF16, tag=f"tch{d}")
                if d in POOL_DIRECT:
                    for h in range(2):
                        psg = g_half(d, h)
                        nc.gpsimd.tensor_mul(
                            tch[:, h * HSL:(h + 1) * HSL], psg[:],
                            XT[:, lo + h * HSL:lo + (h + 1) * HSL])
                else:
                    gbf = tchp.tile([128, SLICE], BF16, tag="gbf", bufs=3)
                    for h in range(2):
                        psg = g_half(d, h)
                        nc.scalar.copy(gbf[:, h * HSL:(h + 1) * HSL], psg[:])
                        nc.vector.tensor_mul(
                            tch[:, h * HSL:(h + 1) * HSL],
                            gbf[:, h * HSL:(h + 1) * HSL],
                            XT[:, lo + h * HSL:lo + (h + 1) * HSL])
                tch_l[d] = tch

            def t2_add(da, db, eng=None):
                for h in range(2):
                    sl = slice(h * HSL, (h + 1) * HSL)
                    e = eng if eng is not None else nc.vector
                    e.tensor_add(tch_l[da][:, sl], tch_l[da][:, sl],
                                 tch_l[db][:, sl])

            # interleave T1 and the T2 tree so adds run as pairs finish
            t1(0); t1(1)
            # pre-load the Sigmoid act table in an Act gap (Copy runs in any
            # set, so copies before/after are unaffected)
            warm2 = small.tile([BC, JE], F32, tag="warm2")
            nc.scalar.activation(warm2[:], vtT[:],
                                 mybir.ActivationFunctionType.Sigmoid)
            t2_add(0, 1)
            t1(2); t1(3)
            t2_add(2, 3)
            t1(4); t1(5)
            t2_add(4, 5, nc.gpsimd); t2_add(0, 2)
            t1(6); t1(7)
            t2_add(6, 7); t2_add(4, 6); t2_add(0, 4)
            Delta = tch_l[0]

            # ---- sigmoid on distinct n only ----
            cbf = tchp.tile([128, SLICE], BF16, tag="cbf")
            nc.scalar.activation(cbf[:, 0:HSL], Delta[:, 0:HSL],
                                 mybir.ActivationFunctionType.Sigmoid)
            nc.scalar.activation(cbf[:, HSL:SLICE], Delta[:, HSL:SLICE],
                                 mybir.ActivationFunctionType.Sigmoid)

            # ---- T4 + s2: ybf_d = cbf * x_d ; A += ybf_d^T Wf_d ----
            ps2 = ps_acc.tile([BC, JE], F32, tag="psacc")

            for d in range(D):
                lo = d * SLICE
                ybf = tch_l[d]  # reuse; Delta (d=0) is dead after sigmoid
                for h in range(2):
                    eng = nc.gpsimd if (d, h) in POOL_T4H else nc.vector
                    eng.tensor_mul(ybf[:, h * HSL:(h + 1) * HSL],
                                   cbf[:, h * HSL:(h + 1) * HSL],
                                   XT[:, lo + h * HSL:lo + (h + 1) * HSL])
                for nb in range(NBT):
                    t = d * NBT + nb
                    nc.tensor.matmul(
                        ps2[:],
                        lhsT=ybf[:, nb * BC:(nb + 1) * BC],
                        rhs=WF[:, t * JE:(t + 1) * JE],
                        start=(t == 0),
                        stop=(t == NT - 1),
                    )

            # ---- ship S early and raw A at the end; assembly host-side ----
            nc.sync.dma_start(sout[:], S[:])
            a2 = small.tile([BC, JE], F32, tag="a2")
            nc.vector.tensor_copy(a2[:], ps2[:])
            nc.sync.dma_start(vout[:], a2[:])

    nc.compile()
    return nc


def _prep_host(x, W):
    bf = ml_dtypes.bfloat16
    # d-major: Wfd[(d,n), je] = W[j, n, e, d]
    Wfd = np.ascontiguousarray(
        np.transpose(W, (3, 1, 0, 2)).reshape(M, JE))
    # wf feed carries the 0.5 iter-1 coupling coefficient
    wf_feed = np.ascontiguousarray(
        (0.5 * Wfd).reshape(NT, 128, JE).transpose(1, 0, 2).reshape(128, FREE)
    ).astype(bf)
    # wft feed: [32 je, m] transposed layout for the 32-contraction g matmuls
    wft_feed = np.ascontiguousarray(Wfd.T).astype(ml_dtypes.float8_e4m3)

    in_maps = []
    for c in range(NCORES):
        xc = x[c * BC:(c + 1) * BC]                 # [32, 6912, 8]
        xd = np.transpose(xc, (2, 1, 0)).reshape(M, BC)   # [(d,n), b]
        xt_feed = np.ascontiguousarray(
            xd.reshape(NT, 128, BC).transpose(1, 0, 2).reshape(128, FREE)
        ).astype(bf)
        in_maps.append({"xt": xt_feed, "wf": wf_feed, "wft": wft_feed})
    return in_maps


def kernel(x, W):
    global _cached
    x = np.asarray(x, dtype=np.float32)
    W = np.asarray(W, dtype=np.float32)
    if _cached is None:
        _cached = _build_program()
    nc = _cached
    in_maps = _prep_host(x, W)
    res = run_bass_kernel_spmd(nc, in_maps, list(range(NCORES)))
    A = np.concatenate(
        [res.results[c]["vout"].reshape(BC, J, E) for c in range(NCORES)],
        axis=0,
    ).astype(np.float64)
    Sf = np.concatenate(
        [res.results[c]["sout"].reshape(BC, J, E) for c in range(NCORES)],
        axis=0,
    ).astype(np.float64)
    s2 = np.empty_like(A)
    s2[:, 0, :] = A[:, 0, :]
    s2[:, 1, :] = Sf[:, 1, :] - A[:, 1, :]
    s2 *= 2.0   # undo the 0.5 folded into the wf feed
    n2 = np.sum(s2 * s2, axis=-1, keepdims=True)
    v = (n2 / (1.0 + n2)) * s2 / np.sqrt(n2 + EPS)
    return v.astype(np.float32)


if __name__ == "__main__":
    import sys
    sys.path.insert(0, "/root/problem")
    import reference as ref
    inputs = ref.setup_inputs()
    expected = np.asarray(ref.reference(**inputs))
    actual = kernel(np.asarray(inputs["x"]), np.asarray(inputs["W"]))
    err = np.abs(actual - expected)
    scale = np.abs(expected).max()
    print("absmax err:", err.max(), "scale:", scale, "rel:", err.max() / scale)
